# revision 3
# baseline (speedup 1.0000x reference)
"""Trainium2 Bass kernel for nn_DecoderLayer (GNN message passing layer).

Strategy: data-parallel over the node axis N=4096 across 8 NeuronCores
(512 nodes/core). All heavy compute runs feature-major ([C, rows] in SBUF)
so every matmul streams >=384-wide moving operands at full fp32r rate with
constant stationary weights. Edge features are pre-transposed/interleaved
on the host so device DMAs are fully contiguous.

Per-core pipeline, per super-block of 32 nodes (1536 edge rows):
  DMA edges (feature-major, f32r)                          -> SBUF
  PE:  z1 = W1e.T @ edgesT (3 K-chunks x 3 col-slices)     -> PSUM
  DVE: z1f = z1 + z1node (per-node broadcast add)          -> SBUF
  ACT: h1 = gelu(z1f + b1)                                 -> SBUF (f32r)
  PE:  z2 = W2.T @ h1                                      -> PSUM
  ACT: h2 = gelu(z2 + b2)                                  -> SBUF (f32r)
  GPS: attn broadcast [1,1536] -> [128,1536]
  DVE: h2a = h2 * attn                                     -> SBUF (f32r)
  PE:  msg = (W3/30).T @ h2a                               -> PSUM
  DVE: agg[:, nodes] = group-sum over k=48 of msg          -> SBUF
Then a small dense phase: x = nodeT + agg + b3*sumA (rank-1 on PE),
LayerNorm (row-major via PE transposes), dense MLP (feature-major),
residual, LayerNorm2, mask, DMA out.
"""

import numpy as np
from contextlib import ExitStack

import concourse.bass as bass
import concourse.bacc as bacc
import concourse.tile as tile
from concourse import mybir
from concourse._compat import with_exitstack
from concourse.bass_utils import run_bass_kernel_spmd

F32 = mybir.dt.float32
F32R = mybir.dt.float32r
GELU = mybir.ActivationFunctionType.Gelu
IDENT = mybir.ActivationFunctionType.Identity
SQRT = mybir.ActivationFunctionType.Sqrt
SQUARE = mybir.ActivationFunctionType.Square
ADD = mybir.AluOpType.add
SUB = mybir.AluOpType.subtract
MULT = mybir.AluOpType.mult
AXX = mybir.AxisListType.X

# Problem constants
N, K, C, ECTX, HID = 4096, 48, 128, 384, 512
NCORES = 8
NN = N // NCORES            # nodes per core = 512
R = NN * K                  # edge rows per core = 24576
SBN = 32                    # nodes per super-block
SBR = SBN * K               # rows per super-block = 1536
NSB = NN // SBN             # super-blocks per core = 16
EPS = 1e-5
SCALE = 30.0


@with_exitstack
def _decoder_kernel(ctx: ExitStack, tc: tile.TileContext, aps: dict):
    nc = tc.nc

    consts = ctx.enter_context(tc.tile_pool(name="consts", bufs=1))

    def load_const(name, shape, dtype):
        t = consts.tile(shape, dtype, tag=name)
        nc.sync.dma_start(t[:], aps[name][:])
        return t

    w1e = load_const("w1e", [128, 3, 128], F32R)
    w1n = load_const("w1n", [128, 128], F32R)
    w2 = load_const("w2", [128, 128], F32R)
    w3 = load_const("w3", [128, 128], F32R)
    wd1 = load_const("wd1", [128, HID], F32R)
    wd2 = load_const("wd2", [128, 4, 128], F32R)
    b1c = load_const("b1c", [128, 1], F32)
    b2c = load_const("b2c", [128, 1], F32)
    b3r = load_const("b3r", [1, 128], F32R)
    bd1 = load_const("bd1", [128, 4], F32)
    bd2 = load_const("bd2", [128, 1], F32)
    g1r = load_const("g1r", [128, 128], F32)
    be1r = load_const("be1r", [128, 128], F32)
    g2r = load_const("g2r", [128, 128], F32)
    be2r = load_const("be2r", [128, 128], F32)
    ident = load_const("ident", [128, 128], F32)
    node_t = load_const("node_t", [128, NN], F32)
    sum_a = load_const("sum_a", [1, NN], F32R)
    mask_t = load_const("mask_t", [128, 4], F32)

    # rounded copy of node features for fp32r matmul input
    node_r = consts.tile([128, NN], F32R, tag="node_r")
    nc.vector.tensor_copy(node_r[:], node_t[:])

    eps_c = consts.tile([128, 1], F32, tag="eps_c")
    nc.vector.memset(eps_c[:], float(EPS))

    # per-node first-layer contribution z1n = W1n.T @ nodeT  [128(C), NN]
    z1n = consts.tile([128, NN], F32, tag="z1n")
    agg = consts.tile([128, NN], F32, tag="agg")

    edges = aps["edges"]
    attn = aps["attn"]

    with (
        tc.tile_pool(name="bigps", bufs=2, space="PSUM") as bigps,
        tc.tile_pool(name="epool", bufs=3) as epool,
        tc.tile_pool(name="apool", bufs=2) as apool,
        tc.tile_pool(name="hpool", bufs=2) as hpool,
    ):
        psz = bigps.tile([128, NN], F32, tag="z1n_ps")
        nc.tensor.matmul(psz[:], w1n[:], node_r[:], start=True, stop=True)
        nc.scalar.copy(z1n[:], psz[:])

        for sb in range(NSB):
            eT = epool.tile([128, 3 * SBR], F32R, tag="eT")
            nc.sync.dma_start(eT[:], edges[:, sb * 3 * SBR:(sb + 1) * 3 * SBR])
            at1 = apool.tile([1, SBR], F32, tag="at1")
            nc.sync.dma_start(at1[:], attn[:, sb * SBR:(sb + 1) * SBR])
            atb = apool.tile([128, SBR], F32, tag="atb")
            nc.gpsimd.partition_broadcast(atb[:], at1[:])

            # ---- message MLP layer 1 (edge part) ----
            ps1 = bigps.tile([128, SBR], F32, tag="big")
            for c in range(3):
                for s in range(3):
                    nc.tensor.matmul(
                        ps1[:, s * 512:(s + 1) * 512],
                        w1e[:, c, :],
                        eT[:, c * SBR + s * 512: c * SBR + (s + 1) * 512],
                        start=(c == 0), stop=(c == 2),
                    )
            # add per-node part (broadcast over k=48)
            z1v = z1n[:, sb * SBN:(sb + 1) * SBN].unsqueeze(2) \
                .broadcast_to([128, SBN, K])
            z1f = hpool.tile([128, SBR], F32, tag="z1f")
            nc.vector.tensor_tensor(
                z1f[:].rearrange("p (n k) -> p n k", k=K),
                ps1[:].rearrange("p (n k) -> p n k", k=K),
                z1v, op=ADD,
            )
            h1 = hpool.tile([128, SBR], F32R, tag="h1")
            nc.scalar.activation(h1[:], z1f[:], GELU, bias=b1c[:, :])

            # ---- layer 2 ----
            ps2 = bigps.tile([128, SBR], F32, tag="big")
            for s in range(3):
                nc.tensor.matmul(
                    ps2[:, s * 512:(s + 1) * 512], w2[:],
                    h1[:, s * 512:(s + 1) * 512], start=True, stop=True,
                )
            h2 = hpool.tile([128, SBR], F32R, tag="h2")
            nc.scalar.activation(h2[:], ps2[:], GELU, bias=b2c[:, :])

            # ---- attention scaling + layer 3 ----
            h2a = hpool.tile([128, SBR], F32R, tag="h2a")
            nc.vector.tensor_tensor(h2a[:], h2[:], atb[:], op=MULT)
            ps3 = bigps.tile([128, SBR], F32, tag="big")
            for s in range(3):
                nc.tensor.matmul(
                    ps3[:, s * 512:(s + 1) * 512], w3[:],
                    h2a[:, s * 512:(s + 1) * 512], start=True, stop=True,
                )
            # ---- aggregate over k ----
            nc.vector.tensor_reduce(
                agg[:, sb * SBN:(sb + 1) * SBN],
                ps3[:].rearrange("p (n k) -> p n k", k=K),
                axis=AXX, op=ADD,
            )

    # ======== dense phase ========
    with (
        tc.tile_pool(name="densps", bufs=6, space="PSUM") as densps,
        tc.tile_pool(name="dpool", bufs=1) as dpool,
        tc.tile_pool(name="small", bufs=1) as small,
    ):
        def transpose4(dst_ps, src_sb):
            for t in range(4):
                nc.tensor.transpose(
                    dst_ps[:, t * 128:(t + 1) * 128],
                    src_sb[:, t * 128:(t + 1) * 128], ident[:],
                )

        def layernorm(x_rm, g_rep, be_rep, out_t):
            """Row-major LN over C=128 (4 node-tiles packed along free dim)."""
            x3 = x_rm[:].rearrange("p (t c) -> p t c", c=128)
            mu = small.tile([128, 4], F32, tag="mu")
            nc.vector.tensor_reduce(mu[:], x3, axis=AXX, op=ADD)
            mu_s = small.tile([128, 4], F32, tag="mu_s")
            nc.vector.tensor_scalar_mul(mu_s[:], mu[:], 1.0 / 128.0)
            xc = dpool.tile([128, NN], F32, tag="xc")
            nc.vector.tensor_tensor(
                xc[:].rearrange("p (t c) -> p t c", c=128), x3,
                mu_s[:].unsqueeze(2).broadcast_to([128, 4, 128]), op=SUB)
            sq = dpool.tile([128, NN], F32, tag="sq")
            nc.scalar.activation(sq[:], xc[:], SQUARE)
            vs = small.tile([128, 4], F32, tag="vs")
            nc.vector.tensor_reduce(
                vs[:], sq[:].rearrange("p (t c) -> p t c", c=128),
                axis=AXX, op=ADD)
            sd = small.tile([128, 4], F32, tag="sd")
            nc.scalar.activation(sd[:], vs[:], SQRT, scale=1.0 / 128.0,
                                 bias=eps_c[:, :])
            rstd = small.tile([128, 4], F32, tag="rstd")
            nc.vector.reciprocal(rstd[:], sd[:])
            xn = dpool.tile([128, NN], F32, tag="xn")
            nc.vector.tensor_tensor(
                xn[:].rearrange("p (t c) -> p t c", c=128),
                xc[:].rearrange("p (t c) -> p t c", c=128),
                rstd[:].unsqueeze(2).broadcast_to([128, 4, 128]), op=MULT)
            xg = dpool.tile([128, NN], F32, tag="xg")
            nc.vector.tensor_tensor(
                xg[:].rearrange("p (t c) -> p t c", c=128),
                xn[:].rearrange("p (t c) -> p t c", c=128),
                g_rep[:].unsqueeze(1).broadcast_to([128, 4, 128]), op=MULT)
            nc.vector.tensor_tensor(
                out_t[:].rearrange("p (t c) -> p t c", c=128),
                xg[:].rearrange("p (t c) -> p t c", c=128),
                be_rep[:].unsqueeze(1).broadcast_to([128, 4, 128]), op=ADD)

        # x = nodeT + agg + outer(b3, sumA)  (feature-major)
        psbx = densps.tile([128, NN], F32, tag="ps")
        nc.tensor.matmul(psbx[:], b3r[:], sum_a[:], start=True, stop=True)
        xt1 = dpool.tile([128, NN], F32, tag="xt1")
        nc.vector.tensor_tensor(xt1[:], node_t[:], agg[:], op=ADD)
        xT = dpool.tile([128, NN], F32, tag="xT")
        nc.vector.tensor_tensor(xT[:], xt1[:], psbx[:], op=ADD)

        # transpose to row-major for LN1
        pst = densps.tile([128, NN], F32, tag="ps")
        transpose4(pst, xT)
        x_rm = dpool.tile([128, NN], F32, tag="x_rm")
        nc.scalar.copy(x_rm[:], pst[:])
        x1n = dpool.tile([128, NN], F32, tag="x1n")
        layernorm(x_rm, g1r, be1r, x1n)

        # back to feature-major for the dense MLP
        pst2 = densps.tile([128, NN], F32, tag="ps")
        transpose4(pst2, x1n)
        x1nT = dpool.tile([128, NN], F32R, tag="x1nT")
        nc.scalar.copy(x1nT[:], pst2[:])

        hd = []
        for j in range(4):
            psd = densps.tile([128, NN], F32, tag="ps")
            nc.tensor.matmul(psd[:], wd1[:, j * 128:(j + 1) * 128], x1nT[:],
                             start=True, stop=True)
            h = dpool.tile([128, NN], F32R, tag=f"hd{j}")
            nc.scalar.activation(h[:], psd[:], GELU, bias=bd1[:, j:j + 1])
            hd.append(h)
        psd2 = densps.tile([128, NN], F32, tag="ps")
        for j in range(4):
            nc.tensor.matmul(psd2[:], wd2[:, j, :], hd[j][:],
                             start=(j == 0), stop=(j == 3))
        dT = dpool.tile([128, NN], F32, tag="dT")
        nc.scalar.activation(dT[:], psd2[:], IDENT, bias=bd2[:, :])

        # residual (row-major) + LN2 + mask
        pst3 = densps.tile([128, NN], F32, tag="ps")
        transpose4(pst3, dT)
        x2 = dpool.tile([128, NN], F32, tag="x2")
        nc.vector.tensor_tensor(x2[:], x1n[:], pst3[:], op=ADD)
        x2n = dpool.tile([128, NN], F32, tag="x2n")
        layernorm(x2, g2r, be2r, x2n)
        o_sb = dpool.tile([128, NN], F32, tag="o_sb")
        nc.vector.tensor_tensor(
            o_sb[:].rearrange("p (t c) -> p t c", c=128),
            x2n[:].rearrange("p (t c) -> p t c", c=128),
            mask_t[:].unsqueeze(2).broadcast_to([128, 4, 128]), op=MULT)
        nc.sync.dma_start(
            aps["out"].rearrange("(t p) c -> p t c", p=128),
            o_sb[:].rearrange("p (t c) -> p t c", c=128))


_CACHE = {}


def _build_program():
    if "nc" in _CACHE:
        return _CACHE["nc"]
    nc = bacc.Bacc("TRN2", target_bir_lowering=False, debug=False)
    aps = {}

    def din(name, shape, dtype):
        aps[name] = nc.dram_tensor(name, shape, dtype, kind="ExternalInput").ap()

    din("edges", [128, NSB * 3 * SBR], F32R)
    din("attn", [1, R], F32)
    din("node_t", [128, NN], F32)
    din("sum_a", [1, NN], F32R)
    din("mask_t", [128, 4], F32)
    din("w1e", [128, 3, 128], F32R)
    din("w1n", [128, 128], F32R)
    din("w2", [128, 128], F32R)
    din("w3", [128, 128], F32R)
    din("wd1", [128, HID], F32R)
    din("wd2", [128, 4, 128], F32R)
    din("b1c", [128, 1], F32)
    din("b2c", [128, 1], F32)
    din("b3r", [1, 128], F32R)
    din("bd1", [128, 4], F32)
    din("bd2", [128, 1], F32)
    din("g1r", [128, 128], F32)
    din("be1r", [128, 128], F32)
    din("g2r", [128, 128], F32)
    din("be2r", [128, 128], F32)
    din("ident", [128, 128], F32)
    aps["out"] = nc.dram_tensor("out", [NN, C], F32, kind="ExternalOutput").ap()

    with tile.TileContext(nc) as tc:
        _decoder_kernel(tc, aps)
    nc.compile()
    _CACHE["nc"] = nc
    return nc


def _prep_shared(W_m1, b_m1, W_m2, b_m2, W_m3, b_m3, g1, beta1,
                 W_d1, b_d1, W_d2, b_d2, g2, beta2):
    f = np.float32
    rep = lambda v: np.ascontiguousarray(np.tile(np.asarray(v, f)[None, :],
                                                 (128, 1)))
    return {
        "w1e": np.ascontiguousarray(
            np.asarray(W_m1, f)[:, C:].T.reshape(3, 128, 128)
            .transpose(1, 0, 2)),
        "w1n": np.ascontiguousarray(np.asarray(W_m1, f)[:, :C].T),
        "w2": np.ascontiguousarray(np.asarray(W_m2, f).T),
        "w3": np.ascontiguousarray((np.asarray(W_m3, f) / SCALE).T),
        "wd1": np.ascontiguousarray(np.asarray(W_d1, f).T),
        "wd2": np.ascontiguousarray(
            np.asarray(W_d2, f).T.reshape(4, 128, 128).transpose(1, 0, 2)),
        "b1c": np.ascontiguousarray(np.asarray(b_m1, f)[:, None]),
        "b2c": np.ascontiguousarray(np.asarray(b_m2, f)[:, None]),
        "b3r": np.ascontiguousarray(np.asarray(b_m3, f)[None, :]),
        "bd1": np.ascontiguousarray(np.asarray(b_d1, f).reshape(4, 128).T),
        "bd2": np.ascontiguousarray(np.asarray(b_d2, f)[:, None]),
        "g1r": rep(g1), "be1r": rep(beta1), "g2r": rep(g2), "be2r": rep(beta2),
        "ident": np.eye(128, dtype=f),
    }


def kernel(node_features, layer_edge_features, mask, attention_mask,
           W_m1, b_m1, W_m2, b_m2, W_m3, b_m3, g1, beta1,
           W_d1, b_d1, W_d2, b_d2, g2, beta2):
    f = np.float32
    node_features = np.asarray(node_features, f)
    layer_edge_features = np.asarray(layer_edge_features, f)
    mask = np.asarray(mask, f)
    attention_mask = np.asarray(attention_mask, f)

    shared = _prep_shared(W_m1, b_m1, W_m2, b_m2, W_m3, b_m3, g1, beta1,
                          W_d1, b_d1, W_d2, b_d2, g2, beta2)

    in_maps = []
    for ci in range(NCORES):
        lo, hi = ci * NN, (ci + 1) * NN
        e = layer_edge_features[lo:hi].reshape(R, ECTX).T  # [384, R]
        edges_il = np.ascontiguousarray(
            e.reshape(3, 128, NSB, SBR).transpose(1, 2, 0, 3)
            .reshape(128, NSB * 3 * SBR))
        am = attention_mask[lo:hi]
        m = {
            "edges": edges_il,
            "attn": np.ascontiguousarray(am.reshape(1, R)),
            "node_t": np.ascontiguousarray(node_features[lo:hi].T),
            "sum_a": np.ascontiguousarray(
                (am.sum(axis=1) / SCALE).reshape(1, NN).astype(f)),
            "mask_t": np.ascontiguousarray(mask[lo:hi].reshape(4, 128).T),
        }
        m.update(shared)
        in_maps.append(m)

    nc = _build_program()
    res = run_bass_kernel_spmd(nc, in_maps, core_ids=list(range(NCORES)))
    out = np.concatenate([res.results[i]["out"] for i in range(NCORES)], axis=0)
    return out.astype(np.float32)


# revision 5
# speedup vs baseline: 1.1651x; 1.1651x over previous
"""Trainium2 Bass kernel for nn_DecoderLayer (GNN message passing layer).

Strategy: data-parallel over the node axis N=4096 across 8 NeuronCores
(512 nodes/core). All heavy compute runs feature-major ([C, rows] in SBUF)
so every matmul streams >=384-wide moving operands at full fp32r rate with
constant stationary weights. Edge features are pre-transposed/interleaved
on the host so device DMAs are fully contiguous.

Per-core pipeline, per super-block of 32 nodes (1536 edge rows):
  DMA edges (feature-major, f32r)                          -> SBUF
  PE:  z1 = W1e.T @ edgesT (3 K-chunks x 3 col-slices)     -> PSUM
  DVE: z1f = z1 + z1node (per-node broadcast add)          -> SBUF
  ACT: h1 = gelu(z1f + b1)                                 -> SBUF (f32r)
  PE:  z2 = W2.T @ h1                                      -> PSUM
  ACT: h2 = gelu(z2 + b2)                                  -> SBUF (f32r)
  GPS: attn broadcast [1,1536] -> [128,1536]
  DVE: h2a = h2 * attn                                     -> SBUF (f32r)
  PE:  msg = (W3/30).T @ h2a                               -> PSUM
  DVE: agg[:, nodes] = group-sum over k=48 of msg          -> SBUF
Then a small dense phase: x = nodeT + agg + b3*sumA (rank-1 on PE),
LayerNorm (row-major via PE transposes), dense MLP (feature-major),
residual, LayerNorm2, mask, DMA out.
"""

import numpy as np
from contextlib import ExitStack

import concourse.bass as bass
import concourse.bacc as bacc
import concourse.tile as tile
from concourse import mybir
from concourse._compat import with_exitstack
from concourse.bass_utils import run_bass_kernel_spmd

F32 = mybir.dt.float32
F32R = mybir.dt.float32r
GELU = mybir.ActivationFunctionType.Gelu
IDENT = mybir.ActivationFunctionType.Identity
SQRT = mybir.ActivationFunctionType.Sqrt
SQUARE = mybir.ActivationFunctionType.Square
ADD = mybir.AluOpType.add
SUB = mybir.AluOpType.subtract
MULT = mybir.AluOpType.mult
AXX = mybir.AxisListType.X

# Problem constants
N, K, C, ECTX, HID = 4096, 48, 128, 384, 512
NCORES = 8
NN = N // NCORES            # nodes per core = 512
R = NN * K                  # edge rows per core = 24576
SBN = 32                    # nodes per super-block
SBR = SBN * K               # rows per super-block = 1536
NSB = NN // SBN             # super-blocks per core = 16
EPS = 1e-5
SCALE = 30.0


@with_exitstack
def _decoder_kernel(ctx: ExitStack, tc: tile.TileContext, aps: dict):
    nc = tc.nc

    consts = ctx.enter_context(tc.tile_pool(name="consts", bufs=1))

    def load_const(name, shape, dtype):
        t = consts.tile(shape, dtype, tag=name)
        nc.sync.dma_start(t[:], aps[name][:])
        return t

    w1e = load_const("w1e", [128, 3, 128], F32R)
    w1n = load_const("w1n", [128, 128], F32R)
    w2 = load_const("w2", [128, 128], F32R)
    w3 = load_const("w3", [128, 128], F32R)
    wd1 = load_const("wd1", [128, HID], F32R)
    wd2 = load_const("wd2", [128, 4, 128], F32R)
    b1c = load_const("b1c", [128, 1], F32)
    b2c = load_const("b2c", [128, 1], F32)
    b3r = load_const("b3r", [1, 128], F32R)
    bd1 = load_const("bd1", [128, 4], F32)
    bd2 = load_const("bd2", [128, 1], F32)
    g1r = load_const("g1r", [128, 128], F32)
    be1r = load_const("be1r", [128, 128], F32)
    g2r = load_const("g2r", [128, 128], F32)
    be2r = load_const("be2r", [128, 128], F32)
    ident = load_const("ident", [128, 128], F32)
    node_t = load_const("node_t", [128, NN], F32)
    sum_a = load_const("sum_a", [1, NN], F32R)
    mask_t = load_const("mask_t", [128, 4], F32)

    # rounded copy of node features for fp32r matmul input
    node_r = consts.tile([128, NN], F32R, tag="node_r")
    nc.vector.tensor_copy(node_r[:], node_t[:])

    eps_c = consts.tile([128, 1], F32, tag="eps_c")
    nc.vector.memset(eps_c[:], float(EPS))

    # per-node first-layer contribution z1n = W1n.T @ nodeT  [128(C), NN]
    z1n = consts.tile([128, NN], F32, tag="z1n")
    agg = consts.tile([128, NN], F32, tag="agg")

    edges = aps["edges"]
    attn = aps["attn"]

    with (
        tc.tile_pool(name="ps1p", bufs=1, space="PSUM") as ps1p,
        tc.tile_pool(name="ps2p", bufs=2, space="PSUM") as ps2p,
        tc.tile_pool(name="ps3p", bufs=1, space="PSUM") as ps3p,
        tc.tile_pool(name="epool", bufs=3) as epool,
        tc.tile_pool(name="apool", bufs=3) as apool,
        tc.tile_pool(name="hpool", bufs=2) as hpool,
    ):
        psz = ps2p.tile([128, NN], F32, tag="sl")
        nc.tensor.matmul(psz[:], w1n[:], node_r[:], start=True, stop=True)
        nc.scalar.copy(z1n[:], psz[:])

        # Software-pipelined main loop: stage A(t) = DMA/broadcast,
        # B(t) = m1, D(t-1) = attn-mult + m3 + aggregate,
        # C(t) = node-add + gelu1 + m2 + gelu2.  D is emitted between
        # B(t) and C(t) so the PE fills its gelu1-wait with m3 work.
        st = {}

        def stageA(t):
            eT = epool.tile([128, 3 * SBR], F32R, tag="eT")
            nc.sync.dma_start(eT[:], edges[:, t * 3 * SBR:(t + 1) * 3 * SBR])
            at1 = apool.tile([1, SBR], F32, tag="at1")
            nc.sync.dma_start(at1[:], attn[:, t * SBR:(t + 1) * SBR])
            atb = apool.tile([128, SBR], F32, tag="atb")
            nc.gpsimd.partition_broadcast(atb[:], at1[:])
            st[t] = {"eT": eT, "atb": atb}

        def stageB(t):
            s_ = st[t]
            eT = s_["eT"]
            ps1 = ps1p.tile([128, SBR], F32, tag="ps1")
            for c in range(3):
                for s in range(3):
                    nc.tensor.matmul(
                        ps1[:, s * 512:(s + 1) * 512],
                        w1e[:, c, :],
                        eT[:, c * SBR + s * 512: c * SBR + (s + 1) * 512],
                        start=(c == 0), stop=(c == 2),
                    )
            s_["ps1"] = ps1

        def stageC(t):
            s_ = st[t]
            ps1 = s_["ps1"]
            z1v = z1n[:, t * SBN:(t + 1) * SBN].unsqueeze(2) \
                .broadcast_to([128, SBN, K])
            z1f = hpool.tile([128, SBR], F32, tag="z1f")
            nc.vector.tensor_tensor(
                z1f[:].rearrange("p (n k) -> p n k", k=K),
                ps1[:].rearrange("p (n k) -> p n k", k=K),
                z1v, op=ADD,
            )
            h1 = hpool.tile([128, SBR], F32R, tag="h1")
            nc.scalar.activation(h1[:], z1f[:], GELU, bias=b1c[:, :])
            h2 = hpool.tile([128, SBR], F32R, tag="h2")
            for s in range(3):
                ps2 = ps2p.tile([128, 512], F32, tag="sl")
                nc.tensor.matmul(ps2[:], w2[:],
                                 h1[:, s * 512:(s + 1) * 512],
                                 start=True, stop=True)
                nc.scalar.activation(h2[:, s * 512:(s + 1) * 512], ps2[:],
                                     GELU, bias=b2c[:, :])
            s_["h2"] = h2

        def stageD(t):
            s_ = st[t]
            h2a = hpool.tile([128, SBR], F32R, tag="h2a")
            nc.vector.tensor_tensor(h2a[:], s_["h2"][:], s_["atb"][:], op=MULT)
            ps3 = ps3p.tile([128, SBR], F32, tag="ps3")
            for s in range(3):
                nc.tensor.matmul(
                    ps3[:, s * 512:(s + 1) * 512], w3[:],
                    h2a[:, s * 512:(s + 1) * 512], start=True, stop=True,
                )
            nc.vector.tensor_reduce(
                agg[:, t * SBN:(t + 1) * SBN],
                ps3[:].rearrange("p (n k) -> p n k", k=K),
                axis=AXX, op=ADD,
            )
            del st[t]

        stageA(0)
        stageA(1)
        for t in range(NSB):
            stageB(t)
            if t >= 1:
                stageD(t - 1)
            if t + 2 < NSB:
                stageA(t + 2)
            stageC(t)
        stageD(NSB - 1)

    # ======== dense phase ========
    with (
        tc.tile_pool(name="densps", bufs=6, space="PSUM") as densps,
        tc.tile_pool(name="dpool", bufs=1) as dpool,
        tc.tile_pool(name="small", bufs=1) as small,
    ):
        def transpose4(dst_ps, src_sb):
            for t in range(4):
                nc.tensor.transpose(
                    dst_ps[:, t * 128:(t + 1) * 128],
                    src_sb[:, t * 128:(t + 1) * 128], ident[:],
                )

        def layernorm(x_rm, g_rep, be_rep, out_t):
            """Row-major LN over C=128 (4 node-tiles packed along free dim)."""
            x3 = x_rm[:].rearrange("p (t c) -> p t c", c=128)
            mu = small.tile([128, 4], F32, tag="mu")
            nc.vector.tensor_reduce(mu[:], x3, axis=AXX, op=ADD)
            mu_s = small.tile([128, 4], F32, tag="mu_s")
            nc.vector.tensor_scalar_mul(mu_s[:], mu[:], 1.0 / 128.0)
            xc = dpool.tile([128, NN], F32, tag="xc")
            nc.vector.tensor_tensor(
                xc[:].rearrange("p (t c) -> p t c", c=128), x3,
                mu_s[:].unsqueeze(2).broadcast_to([128, 4, 128]), op=SUB)
            sq = dpool.tile([128, NN], F32, tag="sq")
            nc.scalar.activation(sq[:], xc[:], SQUARE)
            vs = small.tile([128, 4], F32, tag="vs")
            nc.vector.tensor_reduce(
                vs[:], sq[:].rearrange("p (t c) -> p t c", c=128),
                axis=AXX, op=ADD)
            sd = small.tile([128, 4], F32, tag="sd")
            nc.scalar.activation(sd[:], vs[:], SQRT, scale=1.0 / 128.0,
                                 bias=eps_c[:, :])
            rstd = small.tile([128, 4], F32, tag="rstd")
            nc.vector.reciprocal(rstd[:], sd[:])
            xn = dpool.tile([128, NN], F32, tag="xn")
            nc.vector.tensor_tensor(
                xn[:].rearrange("p (t c) -> p t c", c=128),
                xc[:].rearrange("p (t c) -> p t c", c=128),
                rstd[:].unsqueeze(2).broadcast_to([128, 4, 128]), op=MULT)
            xg = dpool.tile([128, NN], F32, tag="xg")
            nc.vector.tensor_tensor(
                xg[:].rearrange("p (t c) -> p t c", c=128),
                xn[:].rearrange("p (t c) -> p t c", c=128),
                g_rep[:].unsqueeze(1).broadcast_to([128, 4, 128]), op=MULT)
            nc.vector.tensor_tensor(
                out_t[:].rearrange("p (t c) -> p t c", c=128),
                xg[:].rearrange("p (t c) -> p t c", c=128),
                be_rep[:].unsqueeze(1).broadcast_to([128, 4, 128]), op=ADD)

        # x = nodeT + agg + outer(b3, sumA)  (feature-major)
        psbx = densps.tile([128, NN], F32, tag="ps")
        nc.tensor.matmul(psbx[:], b3r[:], sum_a[:], start=True, stop=True)
        xt1 = dpool.tile([128, NN], F32, tag="xt1")
        nc.vector.tensor_tensor(xt1[:], node_t[:], agg[:], op=ADD)
        xT = dpool.tile([128, NN], F32, tag="xT")
        nc.vector.tensor_tensor(xT[:], xt1[:], psbx[:], op=ADD)

        # transpose to row-major for LN1
        pst = densps.tile([128, NN], F32, tag="ps")
        transpose4(pst, xT)
        x_rm = dpool.tile([128, NN], F32, tag="x_rm")
        nc.scalar.copy(x_rm[:], pst[:])
        x1n = dpool.tile([128, NN], F32, tag="x1n")
        layernorm(x_rm, g1r, be1r, x1n)

        # back to feature-major for the dense MLP
        pst2 = densps.tile([128, NN], F32, tag="ps")
        transpose4(pst2, x1n)
        x1nT = dpool.tile([128, NN], F32R, tag="x1nT")
        nc.scalar.copy(x1nT[:], pst2[:])

        hd = []
        for j in range(4):
            psd = densps.tile([128, NN], F32, tag="ps")
            nc.tensor.matmul(psd[:], wd1[:, j * 128:(j + 1) * 128], x1nT[:],
                             start=True, stop=True)
            h = dpool.tile([128, NN], F32R, tag=f"hd{j}")
            nc.scalar.activation(h[:], psd[:], GELU, bias=bd1[:, j:j + 1])
            hd.append(h)
        psd2 = densps.tile([128, NN], F32, tag="ps")
        for j in range(4):
            nc.tensor.matmul(psd2[:], wd2[:, j, :], hd[j][:],
                             start=(j == 0), stop=(j == 3))
        dT = dpool.tile([128, NN], F32, tag="dT")
        nc.scalar.activation(dT[:], psd2[:], IDENT, bias=bd2[:, :])

        # residual (row-major) + LN2 + mask
        pst3 = densps.tile([128, NN], F32, tag="ps")
        transpose4(pst3, dT)
        x2 = dpool.tile([128, NN], F32, tag="x2")
        nc.vector.tensor_tensor(x2[:], x1n[:], pst3[:], op=ADD)
        x2n = dpool.tile([128, NN], F32, tag="x2n")
        layernorm(x2, g2r, be2r, x2n)
        o_sb = dpool.tile([128, NN], F32, tag="o_sb")
        nc.vector.tensor_tensor(
            o_sb[:].rearrange("p (t c) -> p t c", c=128),
            x2n[:].rearrange("p (t c) -> p t c", c=128),
            mask_t[:].unsqueeze(2).broadcast_to([128, 4, 128]), op=MULT)
        nc.sync.dma_start(
            aps["out"].rearrange("(t p) c -> p t c", p=128),
            o_sb[:].rearrange("p (t c) -> p t c", c=128))


_CACHE = {}


def _build_program():
    if "nc" in _CACHE:
        return _CACHE["nc"]
    nc = bacc.Bacc("TRN2", target_bir_lowering=False, debug=False)
    aps = {}

    def din(name, shape, dtype):
        aps[name] = nc.dram_tensor(name, shape, dtype, kind="ExternalInput").ap()

    din("edges", [128, NSB * 3 * SBR], F32R)
    din("attn", [1, R], F32)
    din("node_t", [128, NN], F32)
    din("sum_a", [1, NN], F32R)
    din("mask_t", [128, 4], F32)
    din("w1e", [128, 3, 128], F32R)
    din("w1n", [128, 128], F32R)
    din("w2", [128, 128], F32R)
    din("w3", [128, 128], F32R)
    din("wd1", [128, HID], F32R)
    din("wd2", [128, 4, 128], F32R)
    din("b1c", [128, 1], F32)
    din("b2c", [128, 1], F32)
    din("b3r", [1, 128], F32R)
    din("bd1", [128, 4], F32)
    din("bd2", [128, 1], F32)
    din("g1r", [128, 128], F32)
    din("be1r", [128, 128], F32)
    din("g2r", [128, 128], F32)
    din("be2r", [128, 128], F32)
    din("ident", [128, 128], F32)
    aps["out"] = nc.dram_tensor("out", [NN, C], F32, kind="ExternalOutput").ap()

    with tile.TileContext(nc) as tc:
        _decoder_kernel(tc, aps)
    nc.compile()
    _CACHE["nc"] = nc
    return nc


def _prep_shared(W_m1, b_m1, W_m2, b_m2, W_m3, b_m3, g1, beta1,
                 W_d1, b_d1, W_d2, b_d2, g2, beta2):
    f = np.float32
    rep = lambda v: np.ascontiguousarray(np.tile(np.asarray(v, f)[None, :],
                                                 (128, 1)))
    return {
        "w1e": np.ascontiguousarray(
            np.asarray(W_m1, f)[:, C:].T.reshape(3, 128, 128)
            .transpose(1, 0, 2)),
        "w1n": np.ascontiguousarray(np.asarray(W_m1, f)[:, :C].T),
        "w2": np.ascontiguousarray(np.asarray(W_m2, f).T),
        "w3": np.ascontiguousarray((np.asarray(W_m3, f) / SCALE).T),
        "wd1": np.ascontiguousarray(np.asarray(W_d1, f).T),
        "wd2": np.ascontiguousarray(
            np.asarray(W_d2, f).T.reshape(4, 128, 128).transpose(1, 0, 2)),
        "b1c": np.ascontiguousarray(np.asarray(b_m1, f)[:, None]),
        "b2c": np.ascontiguousarray(np.asarray(b_m2, f)[:, None]),
        "b3r": np.ascontiguousarray(np.asarray(b_m3, f)[None, :]),
        "bd1": np.ascontiguousarray(np.asarray(b_d1, f).reshape(4, 128).T),
        "bd2": np.ascontiguousarray(np.asarray(b_d2, f)[:, None]),
        "g1r": rep(g1), "be1r": rep(beta1), "g2r": rep(g2), "be2r": rep(beta2),
        "ident": np.eye(128, dtype=f),
    }


def kernel(node_features, layer_edge_features, mask, attention_mask,
           W_m1, b_m1, W_m2, b_m2, W_m3, b_m3, g1, beta1,
           W_d1, b_d1, W_d2, b_d2, g2, beta2):
    f = np.float32
    node_features = np.asarray(node_features, f)
    layer_edge_features = np.asarray(layer_edge_features, f)
    mask = np.asarray(mask, f)
    attention_mask = np.asarray(attention_mask, f)

    shared = _prep_shared(W_m1, b_m1, W_m2, b_m2, W_m3, b_m3, g1, beta1,
                          W_d1, b_d1, W_d2, b_d2, g2, beta2)

    in_maps = []
    for ci in range(NCORES):
        lo, hi = ci * NN, (ci + 1) * NN
        e = layer_edge_features[lo:hi].reshape(R, ECTX).T  # [384, R]
        edges_il = np.ascontiguousarray(
            e.reshape(3, 128, NSB, SBR).transpose(1, 2, 0, 3)
            .reshape(128, NSB * 3 * SBR))
        am = attention_mask[lo:hi]
        m = {
            "edges": edges_il,
            "attn": np.ascontiguousarray(am.reshape(1, R)),
            "node_t": np.ascontiguousarray(node_features[lo:hi].T),
            "sum_a": np.ascontiguousarray(
                (am.sum(axis=1) / SCALE).reshape(1, NN).astype(f)),
            "mask_t": np.ascontiguousarray(mask[lo:hi].reshape(4, 128).T),
        }
        m.update(shared)
        in_maps.append(m)

    nc = _build_program()
    res = run_bass_kernel_spmd(nc, in_maps, core_ids=list(range(NCORES)))
    out = np.concatenate([res.results[i]["out"] for i in range(NCORES)], axis=0)
    return out.astype(np.float32)


# revision 6
# speedup vs baseline: 1.2093x; 1.0379x over previous
"""Trainium2 Bass kernel for nn_DecoderLayer (GNN message passing layer).

Strategy: data-parallel over the node axis N=4096 across 8 NeuronCores
(512 nodes/core). All heavy compute runs feature-major ([C, rows] in SBUF)
so every matmul streams >=384-wide moving operands at full fp32r rate with
constant stationary weights. Edge features are pre-transposed/interleaved
on the host so device DMAs are fully contiguous.

Per-core pipeline, per super-block of 32 nodes (1536 edge rows):
  DMA edges (feature-major, f32r)                          -> SBUF
  PE:  z1 = W1e.T @ edgesT (3 K-chunks x 3 col-slices)     -> PSUM
  DVE: z1f = z1 + z1node (per-node broadcast add)          -> SBUF
  ACT: h1 = gelu(z1f + b1)                                 -> SBUF (f32r)
  PE:  z2 = W2.T @ h1                                      -> PSUM
  ACT: h2 = gelu(z2 + b2)                                  -> SBUF (f32r)
  GPS: attn broadcast [1,1536] -> [128,1536]
  DVE: h2a = h2 * attn                                     -> SBUF (f32r)
  PE:  msg = (W3/30).T @ h2a                               -> PSUM
  DVE: agg[:, nodes] = group-sum over k=48 of msg          -> SBUF
Then a small dense phase: x = nodeT + agg + b3*sumA (rank-1 on PE),
LayerNorm (row-major via PE transposes), dense MLP (feature-major),
residual, LayerNorm2, mask, DMA out.
"""

import numpy as np
from contextlib import ExitStack

import concourse.bass as bass
import concourse.bacc as bacc
import concourse.tile as tile
from concourse import mybir
from concourse._compat import with_exitstack
from concourse.bass_utils import run_bass_kernel_spmd

F32 = mybir.dt.float32
F32R = mybir.dt.float32r
GELU = mybir.ActivationFunctionType.Gelu
IDENT = mybir.ActivationFunctionType.Identity
SQRT = mybir.ActivationFunctionType.Sqrt
SQUARE = mybir.ActivationFunctionType.Square
ADD = mybir.AluOpType.add
SUB = mybir.AluOpType.subtract
MULT = mybir.AluOpType.mult
AXX = mybir.AxisListType.X

# Problem constants
N, K, C, ECTX, HID = 4096, 48, 128, 384, 512
NCORES = 8
NN = N // NCORES            # nodes per core = 512
R = NN * K                  # edge rows per core = 24576
SBN = 32                    # nodes per super-block
SBR = SBN * K               # rows per super-block = 1536
NSB = NN // SBN             # super-blocks per core = 16
EPS = 1e-5
SCALE = 30.0


@with_exitstack
def _decoder_kernel(ctx: ExitStack, tc: tile.TileContext, aps: dict):
    nc = tc.nc

    consts = ctx.enter_context(tc.tile_pool(name="consts", bufs=1))

    def load_const(name, shape, dtype):
        t = consts.tile(shape, dtype, tag=name)
        nc.sync.dma_start(t[:], aps[name][:])
        return t

    w1e = load_const("w1e", [128, 3, 128], F32R)
    w1n = load_const("w1n", [128, 128], F32R)
    w2 = load_const("w2", [128, 128], F32R)
    w3 = load_const("w3", [128, 128], F32R)
    wd1 = load_const("wd1", [128, HID], F32R)
    wd2 = load_const("wd2", [128, 4, 128], F32R)
    b1c = load_const("b1c", [128, 1], F32)
    b2c = load_const("b2c", [128, 1], F32)
    b3r = load_const("b3r", [1, 128], F32R)
    bd1 = load_const("bd1", [128, 4], F32)
    bd2 = load_const("bd2", [128, 1], F32)
    g1r = load_const("g1r", [128, 128], F32)
    be1r = load_const("be1r", [128, 128], F32)
    g2r = load_const("g2r", [128, 128], F32)
    be2r = load_const("be2r", [128, 128], F32)
    ident = load_const("ident", [128, 128], F32)
    node_t = load_const("node_t", [128, NN], F32)
    sum_a = load_const("sum_a", [1, NN], F32R)
    mask_t = load_const("mask_t", [128, 4], F32)

    # rounded copy of node features for fp32r matmul input
    node_r = consts.tile([128, NN], F32R, tag="node_r")
    nc.vector.tensor_copy(node_r[:], node_t[:])

    eps_c = consts.tile([128, 1], F32, tag="eps_c")
    nc.vector.memset(eps_c[:], float(EPS))

    agg = consts.tile([128, NN], F32, tag="agg")

    edges = aps["edges"]
    attn = aps["attn"]

    # Deep software pipeline. In period t the engines work on different
    # super-blocks so every cross-engine dependency has ~a full period of
    # slack:  PE: m1(t), m3(t-2), m2(t-1);  ACT: gelu1(t), gelu2(t-1);
    # DVE: attn-mult(t-2), aggregate(t-2);  GpSimd: broadcasts;
    # DMA: edges(t+2).
    with (
        tc.tile_pool(name="slps", bufs=5, space="PSUM") as slps,
        tc.tile_pool(name="ps3p", bufs=1, space="PSUM") as ps3p,
        tc.tile_pool(name="epool", bufs=3) as epool,
        tc.tile_pool(name="a1pool", bufs=3) as a1pool,
        tc.tile_pool(name="abpool", bufs=2) as abpool,
        tc.tile_pool(name="npool", bufs=2) as npool,
        tc.tile_pool(name="hpool", bufs=2) as hpool,
    ):
        st = {}

        def dma_edges(t):
            eT = epool.tile([128, 3 * SBR], F32R, tag="eT")
            nc.sync.dma_start(eT[:], edges[:, t * 3 * SBR:(t + 1) * 3 * SBR])
            st.setdefault(t, {})["eT"] = eT

        def dma_attn(t):
            at1 = a1pool.tile([1, SBR], F32, tag="at1")
            nc.sync.dma_start(at1[:], attn[:, t * SBR:(t + 1) * SBR])
            st.setdefault(t, {})["at1"] = at1

        def make_nrep(t):
            # nodeT slice replicated k=48 times along free dim (for the
            # 4th contraction chunk of m1)
            nrep = npool.tile([128, SBR], F32R, tag="nrep")
            nc.gpsimd.tensor_copy(
                nrep[:].rearrange("p (n k) -> p n k", k=K),
                node_r[:, t * SBN:(t + 1) * SBN].unsqueeze(2)
                .broadcast_to([128, SBN, K]))
            st.setdefault(t, {})["nrep"] = nrep

        def make_atb(t):
            atb = abpool.tile([128, SBR], F32, tag="atb")
            nc.gpsimd.partition_broadcast(atb[:], st[t]["at1"][:])
            st[t]["atb"] = atb

        def stageB(t):
            # m1: 3 edge chunks + node chunk, slice-major with eager gelu1
            s_ = st[t]
            eT, nrep = s_["eT"], s_["nrep"]
            h1 = hpool.tile([128, SBR], F32R, tag="h1")
            for s in range(3):
                ps1 = slps.tile([128, 512], F32, tag="sl")
                for c in range(3):
                    nc.tensor.matmul(
                        ps1[:], w1e[:, c, :],
                        eT[:, c * SBR + s * 512: c * SBR + (s + 1) * 512],
                        start=(c == 0), stop=False)
                nc.tensor.matmul(ps1[:], w1n[:],
                                 nrep[:, s * 512:(s + 1) * 512],
                                 start=False, stop=True)
                nc.scalar.activation(h1[:, s * 512:(s + 1) * 512], ps1[:],
                                     GELU, bias=b1c[:, :])
            s_["h1"] = h1

        def stageC(t):
            s_ = st[t]
            h1 = s_["h1"]
            h2 = hpool.tile([128, SBR], F32R, tag="h2")
            for s in range(3):
                ps2 = slps.tile([128, 512], F32, tag="sl")
                nc.tensor.matmul(ps2[:], w2[:],
                                 h1[:, s * 512:(s + 1) * 512],
                                 start=True, stop=True)
                nc.scalar.activation(h2[:, s * 512:(s + 1) * 512], ps2[:],
                                     GELU, bias=b2c[:, :])
            s_["h2"] = h2

        def stageD(t):
            s_ = st[t]
            h2a = hpool.tile([128, SBR], F32R, tag="h2a")
            nc.vector.tensor_tensor(h2a[:], s_["h2"][:], s_["atb"][:], op=MULT)
            ps3 = ps3p.tile([128, SBR], F32, tag="ps3")
            for s in range(3):
                nc.tensor.matmul(
                    ps3[:, s * 512:(s + 1) * 512], w3[:],
                    h2a[:, s * 512:(s + 1) * 512], start=True, stop=True,
                )
            nc.vector.tensor_reduce(
                agg[:, t * SBN:(t + 1) * SBN],
                ps3[:].rearrange("p (n k) -> p n k", k=K),
                axis=AXX, op=ADD,
            )
            del st[t]

        # prologue
        dma_edges(0)
        dma_attn(0)
        dma_edges(1)
        make_nrep(0)
        for t in range(NSB + 2):
            if 0 <= t - 2:
                make_atb(t - 2)          # gpsimd, feeds mult(t-2) below
            if t + 1 < NSB:
                make_nrep(t + 1)         # gpsimd, feeds m1(t+1)
            if t < NSB:
                stageB(t)                # PE m1 + ACT gelu1
            if 0 <= t - 2:
                stageD(t - 2)            # DVE mult, PE m3, DVE reduce
            if t + 2 < NSB:
                dma_edges(t + 2)
            if t - 1 >= 0 and t - 1 < NSB:
                dma_attn(t - 1) if (t - 1 > 0) else None
            if 0 <= t - 1 < NSB:
                stageC(t - 1)            # PE m2 + ACT gelu2

    # ======== dense phase ========
    with (
        tc.tile_pool(name="densps", bufs=6, space="PSUM") as densps,
        tc.tile_pool(name="dpool", bufs=1) as dpool,
        tc.tile_pool(name="small", bufs=1) as small,
    ):
        def transpose4(dst_ps, src_sb):
            for t in range(4):
                nc.tensor.transpose(
                    dst_ps[:, t * 128:(t + 1) * 128],
                    src_sb[:, t * 128:(t + 1) * 128], ident[:],
                )

        def layernorm(x_rm, g_rep, be_rep, out_t):
            """Row-major LN over C=128 (4 node-tiles packed along free dim)."""
            x3 = x_rm[:].rearrange("p (t c) -> p t c", c=128)
            mu = small.tile([128, 4], F32, tag="mu")
            nc.vector.tensor_reduce(mu[:], x3, axis=AXX, op=ADD)
            mu_s = small.tile([128, 4], F32, tag="mu_s")
            nc.vector.tensor_scalar_mul(mu_s[:], mu[:], 1.0 / 128.0)
            xc = dpool.tile([128, NN], F32, tag="xc")
            nc.vector.tensor_tensor(
                xc[:].rearrange("p (t c) -> p t c", c=128), x3,
                mu_s[:].unsqueeze(2).broadcast_to([128, 4, 128]), op=SUB)
            sq = dpool.tile([128, NN], F32, tag="sq")
            nc.scalar.activation(sq[:], xc[:], SQUARE)
            vs = small.tile([128, 4], F32, tag="vs")
            nc.vector.tensor_reduce(
                vs[:], sq[:].rearrange("p (t c) -> p t c", c=128),
                axis=AXX, op=ADD)
            sd = small.tile([128, 4], F32, tag="sd")
            nc.scalar.activation(sd[:], vs[:], SQRT, scale=1.0 / 128.0,
                                 bias=eps_c[:, :])
            rstd = small.tile([128, 4], F32, tag="rstd")
            nc.vector.reciprocal(rstd[:], sd[:])
            xn = dpool.tile([128, NN], F32, tag="xn")
            nc.vector.tensor_tensor(
                xn[:].rearrange("p (t c) -> p t c", c=128),
                xc[:].rearrange("p (t c) -> p t c", c=128),
                rstd[:].unsqueeze(2).broadcast_to([128, 4, 128]), op=MULT)
            xg = dpool.tile([128, NN], F32, tag="xg")
            nc.vector.tensor_tensor(
                xg[:].rearrange("p (t c) -> p t c", c=128),
                xn[:].rearrange("p (t c) -> p t c", c=128),
                g_rep[:].unsqueeze(1).broadcast_to([128, 4, 128]), op=MULT)
            nc.vector.tensor_tensor(
                out_t[:].rearrange("p (t c) -> p t c", c=128),
                xg[:].rearrange("p (t c) -> p t c", c=128),
                be_rep[:].unsqueeze(1).broadcast_to([128, 4, 128]), op=ADD)

        # x = nodeT + agg + outer(b3, sumA)  (feature-major)
        psbx = densps.tile([128, NN], F32, tag="ps")
        nc.tensor.matmul(psbx[:], b3r[:], sum_a[:], start=True, stop=True)
        xt1 = dpool.tile([128, NN], F32, tag="xt1")
        nc.vector.tensor_tensor(xt1[:], node_t[:], agg[:], op=ADD)
        xT = dpool.tile([128, NN], F32, tag="xT")
        nc.vector.tensor_tensor(xT[:], xt1[:], psbx[:], op=ADD)

        # transpose to row-major for LN1
        pst = densps.tile([128, NN], F32, tag="ps")
        transpose4(pst, xT)
        x_rm = dpool.tile([128, NN], F32, tag="x_rm")
        nc.scalar.copy(x_rm[:], pst[:])
        x1n = dpool.tile([128, NN], F32, tag="x1n")
        layernorm(x_rm, g1r, be1r, x1n)

        # back to feature-major for the dense MLP
        pst2 = densps.tile([128, NN], F32, tag="ps")
        transpose4(pst2, x1n)
        x1nT = dpool.tile([128, NN], F32R, tag="x1nT")
        nc.scalar.copy(x1nT[:], pst2[:])

        hd = []
        for j in range(4):
            psd = densps.tile([128, NN], F32, tag="ps")
            nc.tensor.matmul(psd[:], wd1[:, j * 128:(j + 1) * 128], x1nT[:],
                             start=True, stop=True)
            h = dpool.tile([128, NN], F32R, tag=f"hd{j}")
            nc.scalar.activation(h[:], psd[:], GELU, bias=bd1[:, j:j + 1])
            hd.append(h)
        psd2 = densps.tile([128, NN], F32, tag="ps")
        for j in range(4):
            nc.tensor.matmul(psd2[:], wd2[:, j, :], hd[j][:],
                             start=(j == 0), stop=(j == 3))
        dT = dpool.tile([128, NN], F32, tag="dT")
        nc.scalar.activation(dT[:], psd2[:], IDENT, bias=bd2[:, :])

        # residual (row-major) + LN2 + mask
        pst3 = densps.tile([128, NN], F32, tag="ps")
        transpose4(pst3, dT)
        x2 = dpool.tile([128, NN], F32, tag="x2")
        nc.vector.tensor_tensor(x2[:], x1n[:], pst3[:], op=ADD)
        x2n = dpool.tile([128, NN], F32, tag="x2n")
        layernorm(x2, g2r, be2r, x2n)
        o_sb = dpool.tile([128, NN], F32, tag="o_sb")
        nc.vector.tensor_tensor(
            o_sb[:].rearrange("p (t c) -> p t c", c=128),
            x2n[:].rearrange("p (t c) -> p t c", c=128),
            mask_t[:].unsqueeze(2).broadcast_to([128, 4, 128]), op=MULT)
        nc.sync.dma_start(
            aps["out"].rearrange("(t p) c -> p t c", p=128),
            o_sb[:].rearrange("p (t c) -> p t c", c=128))


_CACHE = {}


def _build_program():
    if "nc" in _CACHE:
        return _CACHE["nc"]
    nc = bacc.Bacc("TRN2", target_bir_lowering=False, debug=False)
    aps = {}

    def din(name, shape, dtype):
        aps[name] = nc.dram_tensor(name, shape, dtype, kind="ExternalInput").ap()

    din("edges", [128, NSB * 3 * SBR], F32R)
    din("attn", [1, R], F32)
    din("node_t", [128, NN], F32)
    din("sum_a", [1, NN], F32R)
    din("mask_t", [128, 4], F32)
    din("w1e", [128, 3, 128], F32R)
    din("w1n", [128, 128], F32R)
    din("w2", [128, 128], F32R)
    din("w3", [128, 128], F32R)
    din("wd1", [128, HID], F32R)
    din("wd2", [128, 4, 128], F32R)
    din("b1c", [128, 1], F32)
    din("b2c", [128, 1], F32)
    din("b3r", [1, 128], F32R)
    din("bd1", [128, 4], F32)
    din("bd2", [128, 1], F32)
    din("g1r", [128, 128], F32)
    din("be1r", [128, 128], F32)
    din("g2r", [128, 128], F32)
    din("be2r", [128, 128], F32)
    din("ident", [128, 128], F32)
    aps["out"] = nc.dram_tensor("out", [NN, C], F32, kind="ExternalOutput").ap()

    with tile.TileContext(nc) as tc:
        _decoder_kernel(tc, aps)
    nc.compile()
    _CACHE["nc"] = nc
    return nc


def _prep_shared(W_m1, b_m1, W_m2, b_m2, W_m3, b_m3, g1, beta1,
                 W_d1, b_d1, W_d2, b_d2, g2, beta2):
    f = np.float32
    rep = lambda v: np.ascontiguousarray(np.tile(np.asarray(v, f)[None, :],
                                                 (128, 1)))
    return {
        "w1e": np.ascontiguousarray(
            np.asarray(W_m1, f)[:, C:].T.reshape(3, 128, 128)
            .transpose(1, 0, 2)),
        "w1n": np.ascontiguousarray(np.asarray(W_m1, f)[:, :C].T),
        "w2": np.ascontiguousarray(np.asarray(W_m2, f).T),
        "w3": np.ascontiguousarray((np.asarray(W_m3, f) / SCALE).T),
        "wd1": np.ascontiguousarray(np.asarray(W_d1, f).T),
        "wd2": np.ascontiguousarray(
            np.asarray(W_d2, f).T.reshape(4, 128, 128).transpose(1, 0, 2)),
        "b1c": np.ascontiguousarray(np.asarray(b_m1, f)[:, None]),
        "b2c": np.ascontiguousarray(np.asarray(b_m2, f)[:, None]),
        "b3r": np.ascontiguousarray(np.asarray(b_m3, f)[None, :]),
        "bd1": np.ascontiguousarray(np.asarray(b_d1, f).reshape(4, 128).T),
        "bd2": np.ascontiguousarray(np.asarray(b_d2, f)[:, None]),
        "g1r": rep(g1), "be1r": rep(beta1), "g2r": rep(g2), "be2r": rep(beta2),
        "ident": np.eye(128, dtype=f),
    }


def kernel(node_features, layer_edge_features, mask, attention_mask,
           W_m1, b_m1, W_m2, b_m2, W_m3, b_m3, g1, beta1,
           W_d1, b_d1, W_d2, b_d2, g2, beta2):
    f = np.float32
    node_features = np.asarray(node_features, f)
    layer_edge_features = np.asarray(layer_edge_features, f)
    mask = np.asarray(mask, f)
    attention_mask = np.asarray(attention_mask, f)

    shared = _prep_shared(W_m1, b_m1, W_m2, b_m2, W_m3, b_m3, g1, beta1,
                          W_d1, b_d1, W_d2, b_d2, g2, beta2)

    in_maps = []
    for ci in range(NCORES):
        lo, hi = ci * NN, (ci + 1) * NN
        e = layer_edge_features[lo:hi].reshape(R, ECTX).T  # [384, R]
        edges_il = np.ascontiguousarray(
            e.reshape(3, 128, NSB, SBR).transpose(1, 2, 0, 3)
            .reshape(128, NSB * 3 * SBR))
        am = attention_mask[lo:hi]
        m = {
            "edges": edges_il,
            "attn": np.ascontiguousarray(am.reshape(1, R)),
            "node_t": np.ascontiguousarray(node_features[lo:hi].T),
            "sum_a": np.ascontiguousarray(
                (am.sum(axis=1) / SCALE).reshape(1, NN).astype(f)),
            "mask_t": np.ascontiguousarray(mask[lo:hi].reshape(4, 128).T),
        }
        m.update(shared)
        in_maps.append(m)

    nc = _build_program()
    res = run_bass_kernel_spmd(nc, in_maps, core_ids=list(range(NCORES)))
    out = np.concatenate([res.results[i]["out"] for i in range(NCORES)], axis=0)
    return out.astype(np.float32)


# revision 7
# speedup vs baseline: 1.6170x; 1.3371x over previous
"""Trainium2 Bass kernel for nn_DecoderLayer (GNN message passing layer).

Strategy: data-parallel over the node axis N=4096 across 8 NeuronCores
(512 nodes/core). All heavy compute runs feature-major ([C, rows] in SBUF)
so every matmul streams >=384-wide moving operands at full fp32r rate with
constant stationary weights. Edge features are pre-transposed/interleaved
on the host so device DMAs are fully contiguous.

Per-core pipeline, per super-block of 32 nodes (1536 edge rows):
  DMA edges (feature-major, f32r)                          -> SBUF
  PE:  z1 = W1e.T @ edgesT (3 K-chunks x 3 col-slices)     -> PSUM
  DVE: z1f = z1 + z1node (per-node broadcast add)          -> SBUF
  ACT: h1 = gelu(z1f + b1)                                 -> SBUF (f32r)
  PE:  z2 = W2.T @ h1                                      -> PSUM
  ACT: h2 = gelu(z2 + b2)                                  -> SBUF (f32r)
  GPS: attn broadcast [1,1536] -> [128,1536]
  DVE: h2a = h2 * attn                                     -> SBUF (f32r)
  PE:  msg = (W3/30).T @ h2a                               -> PSUM
  DVE: agg[:, nodes] = group-sum over k=48 of msg          -> SBUF
Then a small dense phase: x = nodeT + agg + b3*sumA (rank-1 on PE),
LayerNorm (row-major via PE transposes), dense MLP (feature-major),
residual, LayerNorm2, mask, DMA out.
"""

import numpy as np
from contextlib import ExitStack

import concourse.bass as bass
import concourse.bacc as bacc
import concourse.tile as tile
from concourse import mybir
from concourse._compat import with_exitstack
from concourse.bass_utils import run_bass_kernel_spmd

F32 = mybir.dt.float32
F32R = mybir.dt.float32r
GELU = mybir.ActivationFunctionType.Gelu
IDENT = mybir.ActivationFunctionType.Identity
SQRT = mybir.ActivationFunctionType.Sqrt
SQUARE = mybir.ActivationFunctionType.Square
ADD = mybir.AluOpType.add
SUB = mybir.AluOpType.subtract
MULT = mybir.AluOpType.mult
AXX = mybir.AxisListType.X

# Problem constants
N, K, C, ECTX, HID = 4096, 48, 128, 384, 512
NCORES = 8
NN = N // NCORES            # nodes per core = 512
R = NN * K                  # edge rows per core = 24576
SBN = 32                    # nodes per super-block
SBR = SBN * K               # rows per super-block = 1536
NSB = NN // SBN             # super-blocks per core = 16
EPS = 1e-5
SCALE = 30.0


@with_exitstack
def _decoder_kernel(ctx: ExitStack, tc: tile.TileContext, aps: dict):
    nc = tc.nc

    consts = ctx.enter_context(tc.tile_pool(name="consts", bufs=1))

    def load_const(name, shape, dtype):
        t = consts.tile(shape, dtype, tag=name)
        nc.sync.dma_start(t[:], aps[name][:])
        return t

    w1e = load_const("w1e", [128, 3, 128], F32R)
    w1n = load_const("w1n", [128, 128], F32R)
    w2 = load_const("w2", [128, 128], F32R)
    w3 = load_const("w3", [128, 128], F32R)
    wd1 = load_const("wd1", [128, HID], F32R)
    wd2 = load_const("wd2", [128, 4, 128], F32R)
    b1c = load_const("b1c", [128, 1], F32)
    b2c = load_const("b2c", [128, 1], F32)
    b3r = load_const("b3r", [1, 128], F32R)
    bd1 = load_const("bd1", [128, 4], F32)
    bd2 = load_const("bd2", [128, 1], F32)
    g1r = load_const("g1r", [128, 128], F32)
    be1r = load_const("be1r", [128, 128], F32)
    g2r = load_const("g2r", [128, 128], F32)
    be2r = load_const("be2r", [128, 128], F32)
    ident = load_const("ident", [128, 128], F32)
    node_t = load_const("node_t", [128, NN], F32)
    sum_a = load_const("sum_a", [1, NN], F32R)
    mask_t = load_const("mask_t", [128, 4], F32)

    # rounded copy of node features for fp32r matmul input
    node_r = consts.tile([128, NN], F32R, tag="node_r")
    nc.vector.tensor_copy(node_r[:], node_t[:])

    eps_c = consts.tile([128, 1], F32, tag="eps_c")
    nc.vector.memset(eps_c[:], float(EPS))

    agg = consts.tile([128, NN], F32, tag="agg")

    edges = aps["edges"]
    attn = aps["attn"]

    # Deep software pipeline. In period t the engines work on different
    # super-blocks so every cross-engine dependency has ~a full period of
    # slack:  PE: m1(t), m3(t-2), m2(t-1);  ACT: gelu1(t), gelu2(t-1);
    # DVE: attn-mult(t-2), aggregate(t-2);  GpSimd: broadcasts;
    # DMA: edges(t+2).
    with (
        tc.tile_pool(name="slps", bufs=5, space="PSUM") as slps,
        tc.tile_pool(name="ps3p", bufs=1, space="PSUM") as ps3p,
        tc.tile_pool(name="epool", bufs=3) as epool,
        tc.tile_pool(name="a1pool", bufs=3) as a1pool,
        tc.tile_pool(name="abpool", bufs=2) as abpool,
        tc.tile_pool(name="hpool", bufs=2) as hpool,
    ):
        st = {}

        def dma_edges(t):
            eT = epool.tile([128, 3 * SBR], F32R, tag="eT")
            nc.sync.dma_start(eT[:], edges[:, t * 3 * SBR:(t + 1) * 3 * SBR])
            st.setdefault(t, {})["eT"] = eT

        def dma_attn(t):
            at1 = a1pool.tile([1, SBR], F32R, tag="at1")
            nc.sync.dma_start(at1[:], attn[:, t * SBR:(t + 1) * SBR])
            st.setdefault(t, {})["at1"] = at1

        def make_atb(t):
            atb = abpool.tile([128, SBR], F32R, tag="atb")
            nc.gpsimd.partition_broadcast(atb[:], st[t]["at1"][:])
            st[t]["atb"] = atb

        def stageB(t):
            # m1: 3 edge chunks + broadcast node chunk, 384-wide slices
            # (node-aligned: 8 nodes x 48 neighbors per slice)
            s_ = st[t]
            eT = s_["eT"]
            h1 = hpool.tile([128, SBR], F32R, tag="h1")
            for q in range(4):
                ps1 = slps.tile([128, 384], F32, tag="sl")
                for c in range(3):
                    nc.tensor.matmul(
                        ps1[:], w1e[:, c, :],
                        eT[:, c * SBR + q * 384: c * SBR + (q + 1) * 384],
                        start=(c == 0), stop=False)
                nv = node_r[:, t * SBN + q * 8: t * SBN + (q + 1) * 8] \
                    .unsqueeze(2).broadcast_to([128, 8, K])
                nc.tensor.matmul(ps1[:].rearrange("p (n k) -> p n k", k=K),
                                 w1n[:], nv, start=False, stop=True)
                nc.scalar.activation(h1[:, q * 384:(q + 1) * 384], ps1[:],
                                     GELU, bias=b1c[:, :])
            s_["h1"] = h1

        def stageC(t):
            s_ = st[t]
            h1 = s_["h1"]
            h2 = hpool.tile([128, SBR], F32R, tag="h2")
            for s in range(3):
                ps2 = slps.tile([128, 512], F32, tag="sl")
                nc.tensor.matmul(ps2[:], w2[:],
                                 h1[:, s * 512:(s + 1) * 512],
                                 start=True, stop=True)
                nc.scalar.activation(h2[:, s * 512:(s + 1) * 512], ps2[:],
                                     GELU, bias=b2c[:, :])
            s_["h2"] = h2

        def stageD(t):
            s_ = st[t]
            h2a = hpool.tile([128, SBR], F32R, tag="h2a")
            nc.vector.tensor_tensor(h2a[:], s_["h2"][:], s_["atb"][:], op=MULT)
            ps3 = ps3p.tile([128, SBR], F32, tag="ps3")
            for s in range(3):
                nc.tensor.matmul(
                    ps3[:, s * 512:(s + 1) * 512], w3[:],
                    h2a[:, s * 512:(s + 1) * 512], start=True, stop=True,
                )
            nc.vector.tensor_reduce(
                agg[:, t * SBN:(t + 1) * SBN],
                ps3[:].rearrange("p (n k) -> p n k", k=K),
                axis=AXX, op=ADD,
            )
            del st[t]

        # prologue
        dma_edges(0)
        dma_attn(0)
        dma_edges(1)
        for t in range(NSB + 2):
            if 0 <= t - 2:
                make_atb(t - 2)          # gpsimd, feeds mult(t-2) below
            if t < NSB:
                stageB(t)                # PE m1 + ACT gelu1
            if 0 <= t - 2:
                stageD(t - 2)            # DVE mult, PE m3, DVE reduce
            if t + 2 < NSB:
                dma_edges(t + 2)
            if t - 1 >= 0 and t - 1 < NSB:
                dma_attn(t - 1) if (t - 1 > 0) else None
            if 0 <= t - 1 < NSB:
                stageC(t - 1)            # PE m2 + ACT gelu2

    # ======== dense phase ========
    with (
        tc.tile_pool(name="densps", bufs=6, space="PSUM") as densps,
        tc.tile_pool(name="dpool", bufs=1) as dpool,
        tc.tile_pool(name="small", bufs=1) as small,
    ):
        def transpose4(dst_ps, src_sb):
            for t in range(4):
                nc.tensor.transpose(
                    dst_ps[:, t * 128:(t + 1) * 128],
                    src_sb[:, t * 128:(t + 1) * 128], ident[:],
                )

        def layernorm(x_rm, g_rep, be_rep, out_t):
            """Row-major LN over C=128 (4 node-tiles packed along free dim)."""
            x3 = x_rm[:].rearrange("p (t c) -> p t c", c=128)
            mu = small.tile([128, 4], F32, tag="mu")
            nc.vector.tensor_reduce(mu[:], x3, axis=AXX, op=ADD)
            mu_s = small.tile([128, 4], F32, tag="mu_s")
            nc.vector.tensor_scalar_mul(mu_s[:], mu[:], 1.0 / 128.0)
            xc = dpool.tile([128, NN], F32, tag="xc")
            nc.vector.tensor_tensor(
                xc[:].rearrange("p (t c) -> p t c", c=128), x3,
                mu_s[:].unsqueeze(2).broadcast_to([128, 4, 128]), op=SUB)
            sq = dpool.tile([128, NN], F32, tag="sq")
            nc.scalar.activation(sq[:], xc[:], SQUARE)
            vs = small.tile([128, 4], F32, tag="vs")
            nc.vector.tensor_reduce(
                vs[:], sq[:].rearrange("p (t c) -> p t c", c=128),
                axis=AXX, op=ADD)
            sd = small.tile([128, 4], F32, tag="sd")
            nc.scalar.activation(sd[:], vs[:], SQRT, scale=1.0 / 128.0,
                                 bias=eps_c[:, :])
            rstd = small.tile([128, 4], F32, tag="rstd")
            nc.vector.reciprocal(rstd[:], sd[:])
            xn = dpool.tile([128, NN], F32, tag="xn")
            nc.vector.tensor_tensor(
                xn[:].rearrange("p (t c) -> p t c", c=128),
                xc[:].rearrange("p (t c) -> p t c", c=128),
                rstd[:].unsqueeze(2).broadcast_to([128, 4, 128]), op=MULT)
            xg = dpool.tile([128, NN], F32, tag="xg")
            nc.vector.tensor_tensor(
                xg[:].rearrange("p (t c) -> p t c", c=128),
                xn[:].rearrange("p (t c) -> p t c", c=128),
                g_rep[:].unsqueeze(1).broadcast_to([128, 4, 128]), op=MULT)
            nc.vector.tensor_tensor(
                out_t[:].rearrange("p (t c) -> p t c", c=128),
                xg[:].rearrange("p (t c) -> p t c", c=128),
                be_rep[:].unsqueeze(1).broadcast_to([128, 4, 128]), op=ADD)

        # x = nodeT + agg + outer(b3, sumA)  (feature-major)
        psbx = densps.tile([128, NN], F32, tag="ps")
        nc.tensor.matmul(psbx[:], b3r[:], sum_a[:], start=True, stop=True)
        xt1 = dpool.tile([128, NN], F32, tag="xt1")
        nc.vector.tensor_tensor(xt1[:], node_t[:], agg[:], op=ADD)
        xT = dpool.tile([128, NN], F32, tag="xT")
        nc.vector.tensor_tensor(xT[:], xt1[:], psbx[:], op=ADD)

        # transpose to row-major for LN1
        pst = densps.tile([128, NN], F32, tag="ps")
        transpose4(pst, xT)
        x_rm = dpool.tile([128, NN], F32, tag="x_rm")
        nc.scalar.copy(x_rm[:], pst[:])
        x1n = dpool.tile([128, NN], F32, tag="x1n")
        layernorm(x_rm, g1r, be1r, x1n)

        # back to feature-major for the dense MLP
        pst2 = densps.tile([128, NN], F32, tag="ps")
        transpose4(pst2, x1n)
        x1nT = dpool.tile([128, NN], F32R, tag="x1nT")
        nc.scalar.copy(x1nT[:], pst2[:])

        hd = []
        for j in range(4):
            psd = densps.tile([128, NN], F32, tag="ps")
            nc.tensor.matmul(psd[:], wd1[:, j * 128:(j + 1) * 128], x1nT[:],
                             start=True, stop=True)
            h = dpool.tile([128, NN], F32R, tag=f"hd{j}")
            nc.scalar.activation(h[:], psd[:], GELU, bias=bd1[:, j:j + 1])
            hd.append(h)
        psd2 = densps.tile([128, NN], F32, tag="ps")
        for j in range(4):
            nc.tensor.matmul(psd2[:], wd2[:, j, :], hd[j][:],
                             start=(j == 0), stop=(j == 3))
        dT = dpool.tile([128, NN], F32, tag="dT")
        nc.scalar.activation(dT[:], psd2[:], IDENT, bias=bd2[:, :])

        # residual (row-major) + LN2 + mask
        pst3 = densps.tile([128, NN], F32, tag="ps")
        transpose4(pst3, dT)
        x2 = dpool.tile([128, NN], F32, tag="x2")
        nc.vector.tensor_tensor(x2[:], x1n[:], pst3[:], op=ADD)
        x2n = dpool.tile([128, NN], F32, tag="x2n")
        layernorm(x2, g2r, be2r, x2n)
        o_sb = dpool.tile([128, NN], F32, tag="o_sb")
        nc.vector.tensor_tensor(
            o_sb[:].rearrange("p (t c) -> p t c", c=128),
            x2n[:].rearrange("p (t c) -> p t c", c=128),
            mask_t[:].unsqueeze(2).broadcast_to([128, 4, 128]), op=MULT)
        nc.sync.dma_start(
            aps["out"].rearrange("(t p) c -> p t c", p=128),
            o_sb[:].rearrange("p (t c) -> p t c", c=128))


_CACHE = {}


def _build_program():
    if "nc" in _CACHE:
        return _CACHE["nc"]
    nc = bacc.Bacc("TRN2", target_bir_lowering=False, debug=False)
    aps = {}

    def din(name, shape, dtype):
        aps[name] = nc.dram_tensor(name, shape, dtype, kind="ExternalInput").ap()

    din("edges", [128, NSB * 3 * SBR], F32R)
    din("attn", [1, R], F32R)
    din("node_t", [128, NN], F32)
    din("sum_a", [1, NN], F32R)
    din("mask_t", [128, 4], F32)
    din("w1e", [128, 3, 128], F32R)
    din("w1n", [128, 128], F32R)
    din("w2", [128, 128], F32R)
    din("w3", [128, 128], F32R)
    din("wd1", [128, HID], F32R)
    din("wd2", [128, 4, 128], F32R)
    din("b1c", [128, 1], F32)
    din("b2c", [128, 1], F32)
    din("b3r", [1, 128], F32R)
    din("bd1", [128, 4], F32)
    din("bd2", [128, 1], F32)
    din("g1r", [128, 128], F32)
    din("be1r", [128, 128], F32)
    din("g2r", [128, 128], F32)
    din("be2r", [128, 128], F32)
    din("ident", [128, 128], F32)
    aps["out"] = nc.dram_tensor("out", [NN, C], F32, kind="ExternalOutput").ap()

    with tile.TileContext(nc) as tc:
        _decoder_kernel(tc, aps)
    nc.compile()
    _CACHE["nc"] = nc
    return nc


def _prep_shared(W_m1, b_m1, W_m2, b_m2, W_m3, b_m3, g1, beta1,
                 W_d1, b_d1, W_d2, b_d2, g2, beta2):
    f = np.float32
    rep = lambda v: np.ascontiguousarray(np.tile(np.asarray(v, f)[None, :],
                                                 (128, 1)))
    return {
        "w1e": np.ascontiguousarray(
            np.asarray(W_m1, f)[:, C:].T.reshape(3, 128, 128)
            .transpose(1, 0, 2)),
        "w1n": np.ascontiguousarray(np.asarray(W_m1, f)[:, :C].T),
        "w2": np.ascontiguousarray(np.asarray(W_m2, f).T),
        "w3": np.ascontiguousarray((np.asarray(W_m3, f) / SCALE).T),
        "wd1": np.ascontiguousarray(np.asarray(W_d1, f).T),
        "wd2": np.ascontiguousarray(
            np.asarray(W_d2, f).T.reshape(4, 128, 128).transpose(1, 0, 2)),
        "b1c": np.ascontiguousarray(np.asarray(b_m1, f)[:, None]),
        "b2c": np.ascontiguousarray(np.asarray(b_m2, f)[:, None]),
        "b3r": np.ascontiguousarray(np.asarray(b_m3, f)[None, :]),
        "bd1": np.ascontiguousarray(np.asarray(b_d1, f).reshape(4, 128).T),
        "bd2": np.ascontiguousarray(np.asarray(b_d2, f)[:, None]),
        "g1r": rep(g1), "be1r": rep(beta1), "g2r": rep(g2), "be2r": rep(beta2),
        "ident": np.eye(128, dtype=f),
    }


def kernel(node_features, layer_edge_features, mask, attention_mask,
           W_m1, b_m1, W_m2, b_m2, W_m3, b_m3, g1, beta1,
           W_d1, b_d1, W_d2, b_d2, g2, beta2):
    f = np.float32
    node_features = np.asarray(node_features, f)
    layer_edge_features = np.asarray(layer_edge_features, f)
    mask = np.asarray(mask, f)
    attention_mask = np.asarray(attention_mask, f)

    shared = _prep_shared(W_m1, b_m1, W_m2, b_m2, W_m3, b_m3, g1, beta1,
                          W_d1, b_d1, W_d2, b_d2, g2, beta2)

    in_maps = []
    for ci in range(NCORES):
        lo, hi = ci * NN, (ci + 1) * NN
        e = layer_edge_features[lo:hi].reshape(R, ECTX).T  # [384, R]
        edges_il = np.ascontiguousarray(
            e.reshape(3, 128, NSB, SBR).transpose(1, 2, 0, 3)
            .reshape(128, NSB * 3 * SBR))
        am = attention_mask[lo:hi]
        m = {
            "edges": edges_il,
            "attn": np.ascontiguousarray(am.reshape(1, R)),
            "node_t": np.ascontiguousarray(node_features[lo:hi].T),
            "sum_a": np.ascontiguousarray(
                (am.sum(axis=1) / SCALE).reshape(1, NN).astype(f)),
            "mask_t": np.ascontiguousarray(mask[lo:hi].reshape(4, 128).T),
        }
        m.update(shared)
        in_maps.append(m)

    nc = _build_program()
    res = run_bass_kernel_spmd(nc, in_maps, core_ids=list(range(NCORES)))
    out = np.concatenate([res.results[i]["out"] for i in range(NCORES)], axis=0)
    return out.astype(np.float32)
